# revision 7
# baseline (speedup 1.0000x reference)
"""KMeans soft-assignment layer (vq_codebook) for 8x TRN2 NeuronCores.

softmax(-||x-c||^2 / T) over K=512 centroids, T=0.1.

Math: softmax is invariant to the per-row ||x||^2 term, so
logits = (2*x.c - ||c||^2) / T = x @ (20*c)^T - 10*||c||^2.
The -10*||c||^2 row rides the matmul as an extra contraction row against a
ones-row appended to x^T (lhsT = [x^T; 1], rhs = [20*c^T; -10*csq]).

Sharding: data-parallel, batch b -> core b. Each core: 32768 tokens.
Per-core per 128-token tile:
  PE   : fp32 matmul [65,128]^T @ [65,512] -> PSUM logits [128,512]
  DVE  : reduce_max(negate) -> -m [128,1]
  ACT  : exp(logits - m) with fused row-sum accum -> e [128,512], s [128,1]
  DVE  : reciprocal(s) -> r
  DVE/ACT (alternating): e * r -> out tile
  DMA  : out tile -> HBM
"""

import sys

sys.path.insert(0, "/opt/trn_rl_repo")

from contextlib import ExitStack

import numpy as np

import concourse.bacc as bacc
import concourse.bass as bass
import concourse.mybir as mybir
import concourse.tile as tile
from concourse.bass_utils import run_bass_kernel_spmd

N_CORES = 8
B, S, D = 8, 32768, 64
K = 512
TEMP = 0.1
P = 128                      # tokens per tile (partition dim)
N_TILES = S // P             # 256 tiles per core
CHUNK = 16                   # tiles per input DMA chunk
CD = D + 1                   # contraction depth: 64 x-dims + ones row

_NC_CACHE = {}


def _build_nc(repeats=1):
    nc = bacc.Bacc(
        "TRN2", target_bir_lowering=False, debug=False, num_devices=N_CORES
    )
    xt = nc.declare_dram_parameter("xt", [CD, S], mybir.dt.float32, isOutput=False)
    rh = nc.declare_dram_parameter("rh", [CD, K], mybir.dt.float32, isOutput=False)
    out = nc.declare_dram_parameter("out", [S, K], mybir.dt.float32, isOutput=True)

    with tile.TileContext(nc) as tc, ExitStack() as ctx:
        const_pool = ctx.enter_context(tc.tile_pool(name="const", bufs=1))
        in_pool = ctx.enter_context(tc.tile_pool(name="xin", bufs=3))
        psum_pool = ctx.enter_context(tc.tile_pool(name="ps", bufs=3, space="PSUM"))
        e_pool = ctx.enter_context(tc.tile_pool(name="e", bufs=3))
        o_pool = ctx.enter_context(tc.tile_pool(name="o", bufs=4))
        stat_pool = ctx.enter_context(tc.tile_pool(name="stat", bufs=8))

        rhs = const_pool.tile([CD, K], mybir.dt.float32)
        nc.sync.dma_start(rhs[:], rh[:])

        for _rep in range(repeats):
            for c in range(N_TILES // CHUNK):
                xin = in_pool.tile([CD, P * CHUNK], mybir.dt.float32)
                nc.sync.dma_start(
                    xin[:], xt[:, c * P * CHUNK : (c + 1) * P * CHUNK]
                )
                for j in range(CHUNK):
                    t = c * CHUNK + j
                    ps = psum_pool.tile([P, K], mybir.dt.float32)
                    nc.tensor.matmul(
                        ps[:], xin[:, j * P : (j + 1) * P], rhs[:],
                        start=True, stop=True,
                    )
                    nm = stat_pool.tile([P, 1], mybir.dt.float32)
                    nc.vector.tensor_reduce(
                        nm[:], ps[:],
                        axis=mybir.AxisListType.X, op=mybir.AluOpType.max,
                        negate=True,
                    )
                    e = e_pool.tile([P, K], mybir.dt.float32)
                    s = stat_pool.tile([P, 1], mybir.dt.float32)
                    nc.scalar.activation(
                        e[:], ps[:], mybir.ActivationFunctionType.Exp,
                        bias=nm[:], scale=1.0, accum_out=s[:],
                    )
                    r = stat_pool.tile([P, 1], mybir.dt.float32)
                    nc.vector.reciprocal(r[:], s[:])
                    o = o_pool.tile([P, K], mybir.dt.float32)
                    if t % 2 == 0:
                        nc.vector.tensor_scalar_mul(o[:], e[:], r[:])
                    else:
                        nc.scalar.activation(
                            o[:], e[:], mybir.ActivationFunctionType.Copy,
                            scale=r[:],
                        )
                    nc.sync.dma_start(out[t * P : (t + 1) * P, :], o[:])
    nc.compile()
    return nc


def _prep_inputs(x, centroids):
    csq = np.sum(centroids.astype(np.float64) ** 2, axis=1)
    rh = np.empty((CD, K), np.float32)
    rh[0:D] = (2.0 / TEMP) * centroids.T
    rh[D] = (-csq / TEMP).astype(np.float32)
    in_maps = []
    for b in range(N_CORES):
        xt = np.empty((CD, S), np.float32)
        xt[0:D] = x[b].T
        xt[D] = 1.0
        in_maps.append({"xt": np.ascontiguousarray(xt), "rh": rh})
    return in_maps


def kernel(x, centroids):
    x = np.asarray(x)
    centroids = np.asarray(centroids)
    in_maps = _prep_inputs(x, centroids)

    if "nc" not in _NC_CACHE:
        _NC_CACHE["nc"] = _build_nc()
    nc = _NC_CACHE["nc"]

    res = run_bass_kernel_spmd(nc, in_maps, list(range(N_CORES))).results
    out = np.stack([res[b]["out"] for b in range(N_CORES)], axis=0)
    return out.reshape(B, S, K)


if __name__ == "__main__":
    xs = np.random.randn(B, S, D).astype(np.float32)
    cs = np.random.randn(K, D).astype(np.float32)
    o = kernel(xs, cs)
    print(o.shape, o.dtype, o[0, 0, :4])


# revision 13
# speedup vs baseline: 9.0768x; 9.0768x over previous
"""KMeans soft-assignment layer (vq_codebook) for 8x TRN2 NeuronCores.

softmax(-||x-c||^2 / T) over K=512 centroids, T=0.1.

Math: softmax is invariant to the per-row ||x||^2 term, so
logits = (2*x.c - ||c||^2) / T = x @ (20*c)^T - 10*||c||^2.
The -10*||c||^2 row rides the matmul as an extra contraction row against a
ones-row appended to x^T (lhsT = [x^T; 1], rhs = [20*c^T; -10*csq]).

Sharding: data-parallel, batch b -> core b. Each core: 32768 tokens.
Per-core per 128-token tile:
  PE   : fp32 matmul [65,128]^T @ [65,512] -> PSUM logits [128,512]
  DVE  : reduce_max(negate) -> -m [128,1]
  ACT  : exp(logits - m) with fused row-sum accum -> e [128,512], s [128,1]
  DVE  : reciprocal(s) -> r
  DVE/ACT (alternating): e * r -> out tile
  DMA  : out tile -> HBM
"""

import sys

sys.path.insert(0, "/opt/trn_rl_repo")

from contextlib import ExitStack

import numpy as np

import concourse.bacc as bacc
import concourse.bass as bass
import concourse.mybir as mybir
import concourse.tile as tile
from concourse.bass_utils import run_bass_kernel_spmd

N_CORES = 8
B, S, D = 8, 32768, 64
K = 512
TEMP = 0.1
P = 128                      # tokens per tile (partition dim)
N_TILES = S // P             # 256 tiles per core
CHUNK = 16                   # tiles per input DMA chunk
CD = D + 1                   # contraction depth: 64 x-dims + ones row

_NC_CACHE = {}


def _build_nc(
    repeats=1,
    bufs_in=3,
    bufs_ps=3,
    bufs_e=3,
    bufs_o=4,
    chunk=CHUNK,
    act_norm_mod=2,
    mm_dtype=mybir.dt.float32,
):
    """act_norm_mod: tiles with t % act_norm_mod == 0 normalize on DVE,
    the rest on ACT. act_norm_mod=0 -> all on DVE; =1 -> all on ACT."""
    nc = bacc.Bacc(
        "TRN2", target_bir_lowering=False, debug=False, num_devices=N_CORES
    )
    xt = nc.declare_dram_parameter("xt", [CD, S], mm_dtype, isOutput=False)
    rh = nc.declare_dram_parameter("rh", [CD, K], mm_dtype, isOutput=False)
    out = nc.declare_dram_parameter("out", [S, K], mybir.dt.float32, isOutput=True)

    with tile.TileContext(nc) as tc, ExitStack() as ctx:
        const_pool = ctx.enter_context(tc.tile_pool(name="const", bufs=1))
        in_pool = ctx.enter_context(tc.tile_pool(name="xin", bufs=bufs_in))
        psum_pool = ctx.enter_context(
            tc.tile_pool(name="ps", bufs=bufs_ps, space="PSUM")
        )
        e_pool = ctx.enter_context(tc.tile_pool(name="e", bufs=bufs_e))
        o_pool = ctx.enter_context(tc.tile_pool(name="o", bufs=bufs_o))
        stat_pool = ctx.enter_context(tc.tile_pool(name="stat", bufs=8))

        rhs = const_pool.tile([CD, K], mm_dtype)
        nc.sync.dma_start(rhs[:], rh[:])

        for _rep in range(repeats):
            for c in range(N_TILES // chunk):
                xin = in_pool.tile([CD, P * chunk], mm_dtype)
                nc.sync.dma_start(
                    xin[:], xt[:, c * P * chunk : (c + 1) * P * chunk]
                )
                for j in range(chunk):
                    t = c * chunk + j
                    ps = psum_pool.tile([P, K], mybir.dt.float32)
                    nc.tensor.matmul(
                        ps[:], xin[:, j * P : (j + 1) * P], rhs[:],
                        start=True, stop=True,
                    )
                    nm = stat_pool.tile([P, 1], mybir.dt.float32)
                    nc.vector.tensor_reduce(
                        nm[:], ps[:],
                        axis=mybir.AxisListType.X, op=mybir.AluOpType.max,
                        negate=True,
                    )
                    e = e_pool.tile([P, K], mybir.dt.float32)
                    s = stat_pool.tile([P, 1], mybir.dt.float32)
                    nc.scalar.activation(
                        e[:], ps[:], mybir.ActivationFunctionType.Exp,
                        bias=nm[:], scale=1.0, accum_out=s[:],
                    )
                    r = stat_pool.tile([P, 1], mybir.dt.float32)
                    nc.vector.reciprocal(r[:], s[:])
                    o = o_pool.tile([P, K], mybir.dt.float32)
                    on_dve = act_norm_mod == 0 or (
                        act_norm_mod > 1 and t % act_norm_mod == 0
                    )
                    if on_dve:
                        nc.vector.tensor_scalar_mul(o[:], e[:], r[:])
                    else:
                        nc.scalar.activation(
                            o[:], e[:], mybir.ActivationFunctionType.Copy,
                            scale=r[:],
                        )
                    nc.sync.dma_start(out[t * P : (t + 1) * P, :], o[:])
    nc.compile()
    return nc


def _prep_inputs(x, centroids):
    csq = np.sum(centroids.astype(np.float64) ** 2, axis=1)
    rh = np.empty((CD, K), np.float32)
    rh[0:D] = (2.0 / TEMP) * centroids.T
    rh[D] = (-csq / TEMP).astype(np.float32)
    in_maps = []
    for b in range(N_CORES):
        xt = np.empty((CD, S), np.float32)
        xt[0:D] = x[b].T
        xt[D] = 1.0
        in_maps.append({"xt": np.ascontiguousarray(xt), "rh": rh})
    return in_maps


def kernel(x, centroids):
    x = np.asarray(x)
    centroids = np.asarray(centroids)
    in_maps = _prep_inputs(x, centroids)

    if "nc" not in _NC_CACHE:
        _NC_CACHE["nc"] = _build_nc(
            1, bufs_ps=4, bufs_e=4, bufs_o=6, act_norm_mod=0
        )
    nc = _NC_CACHE["nc"]

    res = run_bass_kernel_spmd(nc, in_maps, list(range(N_CORES))).results
    out = np.stack([res[b]["out"] for b in range(N_CORES)], axis=0)
    return out.reshape(B, S, K)


if __name__ == "__main__":
    xs = np.random.randn(B, S, D).astype(np.float32)
    cs = np.random.randn(K, D).astype(np.float32)
    o = kernel(xs, cs)
    print(o.shape, o.dtype, o[0, 0, :4])
